# revision 38
# baseline (speedup 1.0000x reference)
"""Trainium2 Bass kernel for nn_Attention_33921651703853 (sparse_attention).

Data-parallel over batch: B=256 -> 32 batches on each of 8 NeuronCores.
All weights replicated; no collectives.

v2 design (vs baseline): the PE streams moving rows at ~2.33 rows/ns and the
schedule was already ~95% dense, so the wins are fewer streamed rows and
fewer instruction/accumulation-group overheads:
  - q/k projections run as fp8e4m3 DoubleRow matmuls (K=256 per instruction,
    2x bf16 MAC throughput; verified on HW). Weights are pre-scaled x64 on
    host so their 0.02-std mass sits in e4m3's normal range; the 1/64 (and
    the softmax 1/sqrt(D) for q) is folded into the PSUM->SBUF evacuation
    scale. Scores tolerate the fp8 noise because logits are O(0.15) and
    softmax is locally linear there (adds ~0.9% to the final rel err).
  - the final out-projection uses token tiles packed across the 4-batch
    group (656 tokens = 5x128 + 16) instead of per-batch (128+36)x4, cutting
    a third of its output-partition waste. (v cannot be packed the same way:
    AV would need matmul operands at non-64-aligned partition offsets, which
    the PE quadrant rules forbid.)
  - attention is head-outer / batch-inner so the dynamic-MLP, exp, denom
    ones-matmul, and reciprocal each run once per (group, head) over all 4
    batches' columns (656 wide) instead of 4x per head, and is
    software-pipelined one head deep (scores/MLP/exp for head h emitted with
    denominator/AV for head h-1) so PE semaphore waits clear early.
  - next group's projection chains (and the one-group-back final projection)
    are zipped into the attention at single-matmul granularity, keeping big
    512-row matmuls behind every short attention matmul in the PE queue.
Remaining wall-clock is ~653us of row streaming + ~100ns per accumulation-
group switch on the ~2k small attention matmuls (M<=128 forces per-batch
scores/AV), which is structural to this dataflow.
"""

import os
import numpy as np
import ml_dtypes

import concourse.bass as bass
import concourse.bacc as bacc
import concourse.mybir as mybir
import concourse.tile as tile
from concourse import bass_utils

BF16 = mybir.dt.bfloat16
FP8 = mybir.dt.float8e4
F32 = mybir.dt.float32
AF = mybir.ActivationFunctionType
ALU = mybir.AluOpType
DR = mybir.MatmulPerfMode.DoubleRow
nbf16 = ml_dtypes.bfloat16
nfp8 = ml_dtypes.float8_e4m3fn

B, N, D, H, NK, DH = 256, 164, 1024, 8, 100, 128
NCORES = 8
BPC = B // NCORES          # 32 batches per core
GB = 4                     # batches per group
NG = BPC // GB             # 8 groups
XCOLS = GB * N             # 656 tokens per group
MASK_NEG = -1.0e30
WS = 64.0                  # host pre-scale on Wq/Wk before fp8 cast
N1 = N - 128               # 36

# packed token tiles per group for the final projection: 5x128 + 16
TOKTILES = [(0, 128), (128, 128), (256, 128), (384, 128), (512, 128), (640, 16)]
NT = len(TOKTILES)

_CACHE = {}
LAST_EXEC_NS = None


def _install_profile_hook():
    """Make run_bass_kernel_spmd(trace=True) work under axon in this image."""
    import sys as _sys
    import types as _types
    try:
        import antenv  # noqa: F401
        try:
            from antenv.axon_hooks import get_axon_ntff_profile_hook  # noqa: F401
        except ImportError:
            from trn_agent_boot.trn_boot import _ntff_profile_via_ctypes
            hook = _ntff_profile_via_ctypes("/opt/axon/libaxon_pjrt.so")
            mod = _types.ModuleType("antenv.axon_hooks")
            mod._hook = hook
            mod.set_axon_ntff_profile_hook = lambda h: setattr(mod, "_hook", h)
            mod.get_axon_ntff_profile_hook = lambda: mod._hook
            _sys.modules["antenv.axon_hooks"] = mod
            antenv.axon_hooks = mod
        if not getattr(bass_utils, "_upload_patched", False):
            _orig_upload = bass_utils.upload_artifacts

            def _safe_upload(tmpdir):
                try:
                    return _orig_upload(tmpdir)
                except Exception:
                    return tmpdir

            bass_utils.upload_artifacts = _safe_upload
            bass_utils._upload_patched = True
        return True
    except Exception as e:  # pragma: no cover
        print(f"profile hook install failed: {type(e).__name__}: {e}")
        return False


def _build_nc():
    nc = bacc.Bacc("TRN2", target_bir_lowering=False, debug=False)

    # ---- DRAM parameters (per-core shapes) ----
    d_xt = nc.dram_tensor("xt", [NG, 128, 8, XCOLS], BF16, kind="ExternalInput")
    d_xt8 = nc.dram_tensor("xt8", [NG, 128, 4, 2, XCOLS], FP8, kind="ExternalInput")
    d_wqk8 = nc.dram_tensor("wqk8", [128, 4, 2, 2 * D], FP8, kind="ExternalInput")
    d_wv = nc.dram_tensor("wv", [128, 8, D], BF16, kind="ExternalInput")
    d_wout = nc.dram_tensor("wout", [128, 8, D], BF16, kind="ExternalInput")
    d_w1 = nc.dram_tensor("w1", [NK, 50], BF16, kind="ExternalInput")
    d_w2 = nc.dram_tensor("w2", [50, NK], BF16, kind="ExternalInput")
    d_b1 = nc.dram_tensor("b1c", [50, 1], F32, kind="ExternalInput")
    d_b2 = nc.dram_tensor("b2c", [NK, 1], F32, kind="ExternalInput")
    d_boutb = nc.dram_tensor("boutb", [128, D], F32, kind="ExternalInput")
    d_mbt0 = nc.dram_tensor("mbt0", [128, BPC], F32, kind="ExternalInput")
    d_mbt1 = nc.dram_tensor("mbt1", [N1, BPC], F32, kind="ExternalInput")
    d_xiant = nc.dram_tensor("xiant", [NG, NK, GB * NK], BF16, kind="ExternalInput")
    d_y = nc.dram_tensor("y", [NG, XCOLS, D], BF16, kind="ExternalOutput")

    xt_ap = d_xt.ap()
    xt8_ap = d_xt8.ap()
    y_ap = d_y.ap()
    xiant_ap = d_xiant.ap()
    SQ = float((D ** -0.5) / WS)   # q evac scale (softmax scale + fp8 descale)
    SK = float(1.0 / WS)           # k evac scale

    with tile.TileContext(nc) as tc:
        with (
            tc.tile_pool(name="const", bufs=1) as cpool,
            tc.tile_pool(name="xt", bufs=2) as xt_pool,
            tc.tile_pool(name="xt8", bufs=2) as xt8_pool,
            tc.tile_pool(name="xian", bufs=2) as xian_pool,
            tc.tile_pool(name="qk", bufs=2) as qk_pool,
            tc.tile_pool(name="vsb", bufs=2) as v_pool,
            tc.tile_pool(name="stsb", bufs=2) as st_pool,
            tc.tile_pool(name="outT", bufs=2) as outT_pool,
            tc.tile_pool(name="ysb", bufs=2) as y_pool,
            tc.tile_pool(name="probs", bufs=2) as probs_pool,
            tc.tile_pool(name="smallsb", bufs=2) as small_pool,
            tc.tile_pool(name="rbcsb", bufs=1) as rbc_pool,
            tc.tile_pool(name="pproj", bufs=2, space="PSUM") as pp,
            tc.tile_pool(name="pfast", bufs=2, space="PSUM") as pf,
            tc.tile_pool(name="pmlp", bufs=1, space="PSUM") as pm,
            tc.tile_pool(name="pout", bufs=2, space="PSUM") as po,
        ):
            # ---- constants ----
            wqk8_sb = cpool.tile([128, 4, 2, 2 * D], FP8, tag="wqk8")
            wv_sb = cpool.tile([128, 8, D], BF16, tag="wv")
            wout_sb = cpool.tile([128, 8, D], BF16, tag="wout")
            w1_sb = cpool.tile([NK, 50], BF16, tag="w1")
            w2_sb = cpool.tile([50, NK], BF16, tag="w2")
            b1_sb = cpool.tile([50, 1], F32, tag="b1")
            b2_sb = cpool.tile([NK, 1], F32, tag="b2")
            boutb_sb = cpool.tile([128, D], F32, tag="boutb")
            mbt0_sb = cpool.tile([128, BPC], F32, tag="mbt0")
            mbt1_sb = cpool.tile([N1, BPC], F32, tag="mbt1")
            onesm_sb = cpool.tile([128, 128], BF16, tag="onesm")
            nc.vector.memset(onesm_sb[:], 1.0)

            def load_consts():
                for dt in range(8):
                    nc.sync.dma_start(wv_sb[:, dt], d_wv.ap()[:, dt])
                nc.sync.dma_start(w1_sb[:], d_w1.ap()[:, :])
                nc.sync.dma_start(w2_sb[:], d_w2.ap()[:, :])
                nc.sync.dma_start(b1_sb[:], d_b1.ap()[:, :])
                nc.sync.dma_start(b2_sb[:], d_b2.ap()[:, :])
                nc.sync.dma_start(mbt0_sb[:], d_mbt0.ap()[:, :])
                nc.sync.dma_start(mbt1_sb[:], d_mbt1.ap()[:, :])
                for dt in range(8):
                    nc.sync.dma_start(wout_sb[:, dt], d_wout.ap()[:, dt])
                nc.sync.dma_start(boutb_sb[:], d_boutb.ap()[:, :])

            group_tiles = {}

            def start_group(g):
                """DMA group g's inputs, allocate tiles, return a list of
                chain closures (each emits one accumulation chain + evac)."""
                xt8_sb = xt8_pool.tile([128, 4, 2, XCOLS], FP8, tag="xt8")
                for q in range(4):
                    nc.sync.dma_start(xt8_sb[:, q], xt8_ap[g, :, q])
                xt_sb = xt_pool.tile([128, 8, XCOLS], BF16, tag="xt")
                for dt in range(8):
                    nc.sync.dma_start(xt_sb[:, dt], xt_ap[g, :, dt])
                xian_sb = xian_pool.tile([NK, GB, NK], BF16, tag="xian")
                nc.sync.dma_start(xian_sb[:], xiant_ap[g, :, :])
                qkT = qk_pool.tile([128, 16, XCOLS], BF16, tag="qkT")
                v_sb = v_pool.tile([128, GB, 2, D], BF16, tag="v")
                group_tiles[g] = (qkT, v_sb, xian_sb)

                def qk_chain(ct, c0, cw):
                    # q (ct<8) / k (ct>=8) channels tile ct: fp8 DoubleRow over
                    # 4 d-tile pairs; de-scale on evacuation. Yields after each
                    # matmul so the zip can interleave at MM granularity.
                    pt = pp.tile([128, 512], F32, tag="proj")
                    for q in range(4):
                        nc.tensor.matmul(
                            pt[:, :cw],
                            wqk8_sb[:, q, :, ct * 128:(ct + 1) * 128],
                            xt8_sb[:, q, :, c0:c0 + cw],
                            start=(q == 0), stop=(q == 3),
                            perf_mode=DR,
                        )
                        yield
                    nc.scalar.activation(
                        qkT[:, ct, c0:c0 + cw], pt[:, :cw], AF.Copy,
                        scale=(SQ if ct < 8 else SK),
                    )

                def v_chain(b, tt, ch):
                    p0, pw = (0, 128) if tt == 0 else (128, N1)
                    pt = pp.tile([128, 512], F32, tag="proj")
                    for dt in range(8):
                        nc.tensor.matmul(
                            pt[:pw, :],
                            xt_sb[:, dt, b * N + p0:b * N + p0 + pw],
                            wv_sb[:, dt, ch * 512:ch * 512 + 512],
                            start=(dt == 0), stop=(dt == 7),
                        )
                        yield
                    nc.vector.tensor_copy(
                        v_sb[:pw, b, tt, ch * 512:ch * 512 + 512], pt[:pw, :])

                # early: all v chains + q/k ct tiles for heads 0-3 (zipped into
                # the PREVIOUS group's attention). late: q/k for heads 4-7
                # (zipped into THIS group's early attention heads). Entries are
                # (generator-factory, n_matmuls) so the zip can pace by MM.
                early, late = [], []
                for h in range(8):
                    dst = early if h < 4 else late
                    for ct in (h, 8 + h):
                        for c0, cw in ((0, 512), (512, XCOLS - 512)):
                            dst.append((lambda ct=ct, c0=c0, cw=cw:
                                        qk_chain(ct, c0, cw), 4))
                    for k in (2 * h, 2 * h + 1):
                        b, r = divmod(k, 4)
                        tt, ch = divmod(r, 2)
                        early.append((lambda b=b, tt=tt, ch=ch:
                                      v_chain(b, tt, ch), 8))
                return early, late

            def final_chain(outT, g, t):
                """Out-projection for packed token tile t of group g."""
                t0, tw = TOKTILES[t]
                y_sb = y_pool.tile([128, D], BF16, tag="y")
                for ch in range(2):
                    yp = pp.tile([128, 512], F32, tag="proj")
                    for h2 in range(H):
                        nc.tensor.matmul(
                            yp[:tw, :],
                            outT[:, h2, t0:t0 + tw],
                            wout_sb[:, h2, ch * 512:ch * 512 + 512],
                            start=(h2 == 0), stop=(h2 == 7),
                        )
                        yield
                    nc.vector.tensor_add(
                        y_sb[:tw, ch * 512:ch * 512 + 512],
                        yp[:tw, :],
                        boutb_sb[:tw, ch * 512:ch * 512 + 512],
                    )
                nc.sync.dma_start(y_ap[g, t0:t0 + tw, :], y_sb[:tw, :])

            def drive(gen_factories):
                for f in gen_factories:
                    if isinstance(f, tuple):
                        f = f[0]
                    for _ in f():
                        pass

            # prologue: qk weights first (first chains are qk chains), then
            # group 0's inputs, then the remaining constants
            for q in range(4):
                nc.sync.dma_start(wqk8_sb[:, q], d_wqk8.ap()[:, q])
            g0_early, g0_late = start_group(0)
            load_consts()
            drive(g0_early)
            late_pending = g0_late

            pending_final = None   # (outT, g) awaiting out-projection

            for g in range(NG):
                qkT, v_sb, xian_sb = group_tiles.pop(g)
                outT = outT_pool.tile([128, H, XCOLS], BF16, tag="outT")

                # zip queue for this group's attention: this group's own late
                # qk chains first (head h's tile needed by head h), then next
                # group's early chains interleaved with the previous group's
                # final projections. Advanced one MATMUL at a time after each
                # attention matmul, so every short attention matmul has a big
                # projection matmul behind it in the PE queue.
                if g + 1 < NG:
                    nxt_early, nxt_late = start_group(g + 1)
                else:
                    nxt_early, nxt_late = [], []
                zipq = list(late_pending)
                late_pending = nxt_late
                finals = (list(range(NT)) if pending_final is not None else [])
                pf_outT = pending_final[0] if pending_final is not None else None
                pf_g = pending_final[1] if pending_final is not None else None
                fi = 0
                for i, c in enumerate(nxt_early):
                    zipq.append(c)
                    if (i + 1) % 6 == 0 and fi < len(finals):
                        t = finals[fi]
                        zipq.append((lambda t=t: final_chain(pf_outT, pf_g, t), 16))
                        fi += 1
                while fi < len(finals):
                    t = finals[fi]
                    zipq.append((lambda t=t: final_chain(pf_outT, pf_g, t), 16))
                    fi += 1

                # credit pacing: one zip matmul is released per `rate` step
                # sites so supply spreads over the whole group and (almost)
                # every short attention matmul is followed by one big one.
                NSITES = 344.0
                supply = sum(nm for _, nm in zipq)
                zstate = {"qi": 0, "gen": None, "credit": 0.0,
                          "rate": supply / NSITES}

                def step(force=False):
                    zstate["credit"] += (10 ** 9 if force else zstate["rate"])
                    while zstate["credit"] >= 1.0:
                        if zstate["gen"] is None:
                            if zstate["qi"] >= len(zipq):
                                zstate["credit"] = 0.0
                                return
                            zstate["gen"] = zipq[zstate["qi"]][0]()
                            zstate["qi"] += 1
                        try:
                            next(zstate["gen"])
                            zstate["credit"] -= 1.0
                        except StopIteration:
                            zstate["gen"] = None

                # attention, software-pipelined one head deep. Iteration h
                # emits scores(h), then denominator/AV for head h-1, then the
                # dynamic MLP + exp for head h — so every PE matmul's
                # producers (scalar/DVE) ran a full block earlier and its
                # semaphore wait is already clear when the PE reaches it.
                hprev = None
                for h in range(H + 1):
                    if h < H:
                        # ---- scores for 4 batches (transposed, mask bias) ----
                        sT0 = st_pool.tile([128, GB, N], F32, tag="sT0")
                        sT1 = st_pool.tile([N1, GB, N], F32, tag="sT1")
                        for b in range(GB):
                            gb = g * GB + b
                            qof = b * N
                            sp0 = pf.tile([128, N], F32, tag="ps")
                            nc.tensor.matmul(sp0[:], qkT[:, 8 + h, qof:qof + 128],
                                             qkT[:, h, qof:qof + N])
                            step()
                            nc.scalar.activation(sT0[:, b, :], sp0[:], AF.Identity,
                                                 bias=mbt0_sb[:, gb:gb + 1])
                            step()
                            sp1 = pf.tile([128, N], F32, tag="ps")
                            nc.tensor.matmul(sp1[:N1, :],
                                             qkT[:, 8 + h, qof + 128:qof + N],
                                             qkT[:, h, qof:qof + N])
                            step()
                            nc.scalar.activation(sT1[:, b, :], sp1[:N1, :],
                                                 AF.Identity,
                                                 bias=mbt1_sb[:, gb:gb + 1])
                            step()
                    if hprev is not None:
                        hp, pprobs0, pprobs1 = hprev
                        # ---- denominator broadcast via ones-matmul ----
                        rbc = rbc_pool.tile([128, GB, N], F32, tag="rbc")
                        for hf in range(2):
                            dbc = pm.tile([128, 2, N], F32, tag="dbc")
                            nc.tensor.matmul(dbc[:], onesm_sb[:, :],
                                             pprobs0[:, 2 * hf:2 * hf + 2, :],
                                             start=True, stop=False)
                            step()
                            nc.tensor.matmul(dbc[:], onesm_sb[:N1, :],
                                             pprobs1[:, 2 * hf:2 * hf + 2, :],
                                             start=False, stop=True)
                            step()
                            nc.vector.reciprocal_approx_fast(
                                rbc[:, 2 * hf:2 * hf + 2, :], dbc[:])
                            step()
                        # ---- AV; normalize on evacuation ----
                        for b in range(GB):
                            oT = po.tile([128, N], F32, tag="oT")
                            nc.tensor.matmul(
                                oT[:], v_sb[:, b, 0, hp * DH:hp * DH + DH],
                                pprobs0[:, b, :], start=True, stop=False)
                            step()
                            nc.tensor.matmul(
                                oT[:], v_sb[:N1, b, 1, hp * DH:hp * DH + DH],
                                pprobs1[:, b, :], start=False, stop=True)
                            step()
                            nc.vector.tensor_mul(outT[:, hp, b * N:b * N + N],
                                                 oT[:], rbc[:, b, :])
                            step()
                    if h < H:
                        # ---- dynamic MLP, batched over the 4 batches ----
                        raqT = small_pool.tile([NK, GB, NK], BF16, tag="raqT")
                        nc.vector.tensor_scalar(raqT[:], sT0[:NK, :, :NK], 0.0,
                                                None, ALU.max)
                        step()
                        mT = pm.tile([NK, GB, NK], F32, tag="pmlp")
                        nc.tensor.matmul(mT[:50], w1_sb[:, :], raqT[:])
                        step()
                        h1T = small_pool.tile([50, GB, NK], BF16, tag="h1T")
                        nc.vector.tensor_scalar(h1T[:], mT[:50], b1_sb[:], 0.0,
                                                ALU.add, ALU.max)
                        step()
                        nc.tensor.matmul(mT[:], w2_sb[:, :], h1T[:])
                        step()
                        lvT = small_pool.tile([NK, GB, NK], BF16, tag="lvT")
                        nc.vector.tensor_scalar(lvT[:], mT[:], b2_sb[:], 0.0,
                                                ALU.add, ALU.max)
                        step()
                        # xian*lv reuses raqT (done being read by m1); bf16 is
                        # plenty: the product is O(0.1) added onto O(1) logits.
                        nc.vector.tensor_mul(raqT[:], xian_sb[:], lvT[:])
                        step()
                        nc.vector.tensor_add(sT0[:NK, :, :NK], sT0[:NK, :, :NK],
                                             raqT[:])
                        step()
                        # ---- exp (no max subtraction; logits are O(1)) ----
                        probs0 = probs_pool.tile([128, GB, N], BF16, tag="p0")
                        nc.scalar.activation(probs0[:], sT0[:], AF.Exp)
                        step()
                        probs1 = probs_pool.tile([N1, GB, N], BF16, tag="p1")
                        nc.scalar.activation(probs1[:], sT1[:], AF.Exp)
                        step()
                    hprev = (h, probs0, probs1) if h < H else None

                step(force=True)   # drain any remainder of the zip queue
                pending_final = (outT, g)

            drive(lambda t=t: final_chain(pending_final[0], pending_final[1], t)
                  for t in range(NT))

    nc.compile()
    return nc


def _prep_core_inputs(xc, maskc, xianc, shared):
    # xT tiles: [BPC,N,D] -> (g, p, dt, b, n)
    xt5 = xc.transpose(0, 2, 1).reshape(NG, GB, 8, 128, N)
    xt5 = np.ascontiguousarray(xt5.transpose(0, 3, 2, 1, 4))  # [NG,128,8,GB,N]
    xt = xt5.reshape(NG, 128, 8, XCOLS).astype(nbf16)
    xt8 = xt5.reshape(NG, 128, 4, 2, XCOLS).astype(nfp8)
    # mask bias transposed: [164, BPC]
    mb = np.where(maskc, np.float32(MASK_NEG), np.float32(0.0)).astype(np.float32)
    mbt = np.ascontiguousarray(mb.T)
    # xianT: (g, j, b, i)
    xiant = xianc.transpose(0, 2, 1).reshape(NG, GB, NK, NK)
    xiant = np.ascontiguousarray(xiant.transpose(0, 2, 1, 3)).reshape(NG, NK, GB * NK)
    xiant = xiant.astype(nbf16)
    out = {
        "xt": xt,
        "xt8": xt8,
        "mbt0": np.ascontiguousarray(mbt[:128]),
        "mbt1": np.ascontiguousarray(mbt[128:]),
        "xiant": xiant,
    }
    out.update(shared)
    return out


def kernel(x, mask, xian, Wqkv, W1, b1, W2, b2, Wout, bout):
    global LAST_EXEC_NS
    x = np.asarray(x, dtype=np.float32)
    mask = np.asarray(mask)
    xian = np.asarray(xian, dtype=np.float32)
    Wqkv = np.asarray(Wqkv, dtype=np.float32)
    W1 = np.asarray(W1, dtype=np.float32)
    b1 = np.asarray(b1, dtype=np.float32)
    W2 = np.asarray(W2, dtype=np.float32)
    b2 = np.asarray(b2, dtype=np.float32)
    Wout = np.asarray(Wout, dtype=np.float32)
    bout = np.asarray(bout, dtype=np.float32)

    if "nc" not in _CACHE:
        _CACHE["nc"] = _build_nc()
    nc = _CACHE["nc"]

    # ---- shared weight prep ----
    wqk = Wqkv[:, :2 * D] * np.float32(WS)                     # [1024, 2048]
    wqk8 = np.ascontiguousarray(
        wqk.reshape(4, 2, 128, 2 * D).transpose(2, 0, 1, 3)).astype(nfp8)
    wv_h = np.ascontiguousarray(
        Wqkv[:, 2 * D:].reshape(8, 128, D).transpose(1, 0, 2)).astype(nbf16)
    wout_h = np.ascontiguousarray(
        Wout.reshape(8, 128, D).transpose(1, 0, 2)).astype(nbf16)
    shared = {
        "wqk8": wqk8,
        "wv": wv_h,
        "wout": wout_h,
        "w1": W1.astype(nbf16),
        "w2": W2.astype(nbf16),
        "b1c": np.ascontiguousarray(b1.reshape(50, 1)),
        "b2c": np.ascontiguousarray(b2.reshape(NK, 1)),
        "boutb": np.ascontiguousarray(np.broadcast_to(bout, (128, D))).astype(np.float32),
    }

    in_maps = []
    for c in range(NCORES):
        sl = slice(c * BPC, (c + 1) * BPC)
        in_maps.append(_prep_core_inputs(x[sl], mask[sl], xian[sl], shared))

    trace = bool(int(os.environ.get("KERNEL_TRACE", "0")))
    if trace:
        trace = _install_profile_hook()
    res = bass_utils.run_bass_kernel_spmd(
        nc, in_maps, core_ids=list(range(NCORES)), trace=trace)
    LAST_EXEC_NS = res.exec_time_ns

    out = np.empty((B, N, D), dtype=np.float32)
    for c in range(NCORES):
        out[c * BPC:(c + 1) * BPC] = (
            res.results[c]["y"].reshape(BPC, N, D).astype(np.float32))
    return out
